# revision 29
# baseline (speedup 1.0000x reference)
"""MelSpectrogramNet on 8 TRN2 NeuronCores (Bass/Tile), data-parallel over batch.

Math (per batch item):
  stft[f,t]  = (sum_k x[256t+k]*wc[f,k])^2 + (sum_k x[256t+k]*ws[f,k])^2
  mel        = mel_w @ stft
  x_db       = 10*log10(max(mel, 1e-10));  x_db = max(x_db, max_all(x_db)-80)
  out        = (x_db + 25) / 80

Folded DFT (the key trick): the hann window is exactly symmetric
(w[k] = w[2047-k]), so with j = k - 1023.5 the windowed DFT row is
w*cos(theta_f*j + phi_f). Folding x about the window center into
  e_m(t) = x[256t+1024+m] + x[256t+1023-m]
  o_m(t) = x[256t+1024+m] - x[256t+1023-m]        (m in [0,1024))
gives  cosDFT = cos(phi)C - sin(phi)S,  sinDFT = sin(phi)C + cos(phi)S with
  C_f = sum_m W~c[m,f] e_m,   S_f = sum_m W~s[m,f] o_m
and the power is phi-free:  power_f = C_f^2 + S_f^2.
=> the tensor-engine contraction halves (K=1024 per transform instead of
2x K=2048), which matters because the PE is the bottleneck (GPIO power
throttle caps it at 13/16 duty; the f32r version already ran at ~96% of
that throttled roofline).

Device mapping:
  - x is de-interleaved by 128-column parity into C2[r, par, u] =
    x[256u+128par+r] plus a partition-reversed copy C2R[r,...] =
    C2[127-r,...]; the DVE then computes each 128-row m-chunk of e/o as a
    single tensor_tensor add/sub of two contiguous slices (hidden under
    the matmuls of the previous tile).
  - all matmul operands are bf16 (measured end-to-end rel err ~5e-3 vs
    the 2e-2 gate); PSUM accumulation is fp32.
  - Nyquist: C_1024 = 0 exactly and the S-weight column for f=0 is exactly
    zero, so the S weights carry w~*(-1)^m (the Nyquist sine row) in the
    f=0 slot. Then stft[0] = C_0^2 + S_nyq^2; the mel weight column for
    f=0 is swapped to mel_w[:,1024] and a K=1 rank-1 matmul with
    (mel_w[:,0]-mel_w[:,1024]) x C_0^2 repairs the difference.
  - top_db clamp in linear space: pass 1 keeps out_pre in SBUF and the
    per-core max of mel; after gpsimd partition_all_reduce +
    AllReduce(max), the fixup applies out = max(out_pre, o_thr) in-place
    and DMAs straight out — no DRAM round-trip in the tail.
"""
import sys

sys.path.insert(0, "/opt/trn_rl_repo")

import ml_dtypes
import numpy as np

from concourse import bacc, bass_isa, mybir, tile
from concourse.bass_utils import run_bass_kernel_spmd

dt = mybir.dt
AF = mybir.ActivationFunctionType
ALU = mybir.AluOpType

NCORES = 8
B, T = 32, 221184
WIN, HOP = 2048, 256
FRAMES = (T - WIN) // HOP + 1  # 857
NMEL = 128
BPC = B // NCORES  # 4
UCOLS = T // 256  # 864 columns of 128 per parity
NFC = 8  # f-chunks of 128 (f = 0..1023); f=1024 (Nyquist) folded into S f=0
NMC = 8  # m-chunks of 128 (folded window half, m = 0..1023)
# Second tile overlaps the first by 3 frames so its width is a multiple of 4;
# overlapped frames are recomputed with identical values.
T_TILES = [(0, 512), (FRAMES - 348, 348)]
FIX_TILES = [(0, 512), (512, FRAMES - 512)]  # non-overlapping, for the fixup
C_LOG = 10.0 / float(np.log(10.0))  # 10*log10(x) = C_LOG * ln(x)
AMIN = 1e-10
TOPDB_LIN = 1e-8  # 10**(-80/10)

_compiled = {}


def _build_nc():
    nc = bacc.Bacc(
        "TRN2", target_bir_lowering=False, debug=False, num_devices=NCORES
    )

    c2_d = nc.dram_tensor("c2", [BPC, 128, 2, UCOLS], dt.bfloat16, kind="ExternalInput")
    c2r_d = nc.dram_tensor(
        "c2r", [BPC, 128, 2, UCOLS], dt.bfloat16, kind="ExternalInput"
    )
    wc_d = nc.dram_tensor("wc", [128, NFC, NMC, 128], dt.bfloat16, kind="ExternalInput")
    ws_d = nc.dram_tensor("ws", [128, NFC, NMC, 128], dt.bfloat16, kind="ExternalInput")
    melT_d = nc.dram_tensor("melT", [128, NFC, NMEL], dt.bfloat16, kind="ExternalInput")
    melnyq_d = nc.dram_tensor("melnyq", [1, NMEL], dt.bfloat16, kind="ExternalInput")
    out1_d = nc.dram_tensor("out1", [BPC, NMEL, 512], dt.bfloat16, kind="ExternalOutput")
    out2_d = nc.dram_tensor(
        "out2", [BPC, NMEL, FRAMES - 512], dt.bfloat16, kind="ExternalOutput"
    )

    with tile.TileContext(nc) as tc:
        with (
            tc.tile_pool(name="sbw", bufs=1) as sbw,
            tc.tile_pool(name="sbeo", bufs=2) as sbeo,
            tc.tile_pool(name="sbe", bufs=3) as sbe,
            tc.tile_pool(name="sbf", bufs=8) as sbf,
            tc.tile_pool(name="sbc", bufs=8) as sbc,
            tc.tile_pool(name="psCS", bufs=3, space="PSUM") as psCS,
            tc.tile_pool(name="psM", bufs=2, space="PSUM") as psM,
            tc.tile_pool(name="dram", bufs=1, space="DRAM") as dram,
        ):
            # persistent SBUF tensors
            c2s, c2rs, outp = [], [], []
            for b in range(BPC):
                c2s.append(sbw.tile([128, 2, UCOLS], dt.bfloat16, name=f"c2_{b}"))
                c2rs.append(sbw.tile([128, 2, UCOLS], dt.bfloat16, name=f"c2r_{b}"))
                outp.append(sbw.tile([128, FRAMES], dt.float32, name=f"outp_{b}"))
            wc_t = [sbw.tile([128, NMC, 128], dt.bfloat16, name=f"wc{fc}") for fc in range(NFC)]
            ws_t = [sbw.tile([128, NMC, 128], dt.bfloat16, name=f"ws{fc}") for fc in range(NFC)]
            melT_t = sbw.tile([128, NFC, NMEL], dt.bfloat16, name="melT_t")
            melnyq_t = sbw.tile([1, NMEL], dt.bfloat16, name="melnyq_t")
            nslots = BPC * len(T_TILES)
            maxslots = sbw.tile([128, nslots], dt.float32, name="maxslots")


            # ---- input DMAs: b=0 slices needed by the first tile go first.
            # fold mc=0 needs c2 parity 0 + c2r parity 1, so those two land
            # first.
            nc.gpsimd.dma_start(c2s[0][:, 0, 0:520], c2_d.ap()[0][:, 0, 0:520])
            nc.sync.dma_start(c2rs[0][:, 1, 0:520], c2r_d.ap()[0][:, 1, 0:520])
            nc.gpsimd.dma_start(c2s[0][:, 1, 0:520], c2_d.ap()[0][:, 1, 0:520])
            nc.gpsimd.dma_start(c2rs[0][:, 0, 0:520], c2r_d.ap()[0][:, 0, 0:520])
            # fc=0/1 weights split across sync/scalar queues so the first
            # matmuls are never DMA-starved; melT's first chunks land early
            # (the mel matmul is on the in-order PE queue — starving it
            # stalls the PE), then the remaining f-chunks alternate queues.
            nc.sync.dma_start(wc_t[0][:, 0:4], wc_d.ap()[:, 0, 0:4])
            nc.scalar.dma_start(wc_t[0][:, 4:], wc_d.ap()[:, 0, 4:])
            nc.sync.dma_start(wc_t[1][:], wc_d.ap()[:, 1])
            nc.scalar.dma_start(wc_t[2][:], wc_d.ap()[:, 2])

            # Warm up the collective engine while the DFT runs so the real
            # AllReduce at the end starts with rings already configured.
            ccw_in = dram.tile([1, 128], dt.float32, name="ccw_in")
            ccw_out = dram.tile([1, 128], dt.float32, name="ccw_out")
            nc.gpsimd.collective_compute(
                "AllReduce",
                ALU.max,
                replica_groups=[list(range(NCORES))],
                ins=[ccw_in[:].opt()],
                outs=[ccw_out[:].opt()],
            )

            nc.sync.dma_start(wc_t[3][:], wc_d.ap()[:, 3])
            nc.scalar.dma_start(wc_t[4][:], wc_d.ap()[:, 4])
            nc.sync.dma_start(wc_t[5][:], wc_d.ap()[:, 5])
            nc.scalar.dma_start(wc_t[6][:], wc_d.ap()[:, 6])
            nc.sync.dma_start(wc_t[7][:], wc_d.ap()[:, 7])
            nc.scalar.dma_start(melnyq_t[:], melnyq_d.ap())
            nc.scalar.dma_start(ws_t[0][:], ws_d.ap()[:, 0])
            nc.sync.dma_start(ws_t[1][:], ws_d.ap()[:, 1])
            nc.scalar.dma_start(melT_t[:, 0:2], melT_d.ap()[:, 0:2])
            nc.scalar.dma_start(ws_t[2][:], ws_d.ap()[:, 2])
            nc.sync.dma_start(ws_t[3][:], ws_d.ap()[:, 3])
            nc.scalar.dma_start(melT_t[:, 2:], melT_d.ap()[:, 2:])
            nc.scalar.dma_start(ws_t[4][:], ws_d.ap()[:, 4])
            nc.sync.dma_start(ws_t[5][:], ws_d.ap()[:, 5])
            nc.scalar.dma_start(ws_t[6][:], ws_d.ap()[:, 6])
            nc.sync.dma_start(ws_t[7][:], ws_d.ap()[:, 7])
            nc.gpsimd.dma_start(c2s[0][:, :, 520:], c2_d.ap()[0][:, :, 520:])
            nc.gpsimd.dma_start(c2rs[0][:, :, 520:], c2r_d.ap()[0][:, :, 520:])
            for b in range(1, BPC):
                nc.gpsimd.dma_start(c2s[b][:], c2_d.ap()[b])
                nc.gpsimd.dma_start(c2rs[b][:], c2r_d.ap()[b])

            # ---- pass 1: fold + folded DFT power + mel + log/affine ----
            slots = [(b, t0, tt) for b in range(BPC) for t0, tt in T_TILES]
            # even m-chunks need (c2 par0, c2r par1); odds the other pair —
            # process evens first so the first matmuls match DMA arrival order
            MC_ORDER = [0, 2, 4, 6, 1, 3, 5, 7]

            def emit_fold(si):
                # DVE fold: e/o m-chunks as adds/subs of shifted slices.
                # Slot 0 folds in 256-wide halves so work can start as soon
                # as the first x quarters land (keeps early PE gaps under
                # the ~3.4us HAM re-throttle window).
                b, t0, tt = slots[si]
                e_t = sbeo.tile([128, NMC, tt], dt.bfloat16, tag="e")
                o_t = sbeo.tile([128, NMC, tt], dt.bfloat16, tag="o")
                for mc in MC_ORDER:
                    p1 = mc % 2
                    u1 = t0 + 4 + mc // 2
                    p2 = 1 - p1
                    u2 = t0 + 3 - mc // 2
                    a = c2s[b][:, p1, u1 : u1 + tt]
                    r = c2rs[b][:, p2, u2 : u2 + tt]
                    nc.vector.tensor_tensor(e_t[:, mc], a, r, ALU.add)
                    nc.vector.tensor_tensor(o_t[:, mc], a, r, ALU.subtract)
                return e_t, o_t

            def emit_epilogue(slot, defer=None):
                # mel -> per-slot max -> clamp(AMIN) -> ln -> affine -> outp
                b, t0, tt = slots[slot]
                mel_ps = mel_pss[slot]
                mel_sb = sbe.tile([128, tt], dt.float32, tag="melsb")
                nc.vector.tensor_reduce(
                    maxslots[:, slot : slot + 1], mel_ps[:],
                    mybir.AxisListType.X, ALU.max,
                )
                if defer is not None:
                    defer()  # last slot: thr chain ahead of the Ln/affine
                nc.vector.tensor_scalar(mel_sb[:], mel_ps[:], AMIN, None, ALU.max)
                nc.scalar.activation(mel_sb[:], mel_sb[:], AF.Ln)
                nc.vector.tensor_scalar(
                    outp[b][:, t0 : t0 + tt], mel_sb[:],
                    C_LOG / 80.0, 25.0 / 80.0, ALU.mult, ALU.add,
                )

            eo_next = emit_fold(0)
            mel_pss = {}
            for slot, (b, t0, tt) in enumerate(slots):
                e_t, o_t = eo_next
                mel_ps = psM.tile([128, tt], dt.float32, tag="mel")
                mel_pss[slot] = mel_ps
                # mel matmuls are emitted one fc-iteration late so the
                # in-order PE queue never waits on the Square/add chain;
                # (stft tile, fc) pending between iterations:
                pend = None
                if slot == 0:
                    # While weights are still streaming in, one fc step of
                    # the interleaved loop eats 512KB (wc+ws) per ~4.3us —
                    # more than two DMA queues deliver. Run ALL C transforms
                    # first (only the wc stream gates the PE), banking csq in
                    # SBUF; the ws stream lands during the C phase.
                    csqs = []
                    for fc in range(NFC):
                        c_ps = psCS.tile([128, tt], dt.float32, tag="C")
                        for i, mc in enumerate(MC_ORDER):
                            nc.tensor.matmul(
                                c_ps[:], wc_t[fc][:, mc, :], e_t[:, mc],
                                start=(i == 0), stop=(i == NMC - 1),
                                skip_group_check=True,
                            )
                        csq = sbc.tile([128, tt], dt.bfloat16, tag="csq0")
                        nc.scalar.activation(csq[:], c_ps[:], AF.Square)
                        csqs.append(csq)
                    prev_csq = csqs[0]
                    for fc in range(NFC):
                        if fc == 4:
                            eo_next = emit_fold(1)
                        s_ps = psCS.tile([128, tt], dt.float32, tag="S")
                        for i, mc in enumerate(MC_ORDER):
                            nc.tensor.matmul(
                                s_ps[:], ws_t[fc][:, mc, :], o_t[:, mc],
                                start=(i == 0), stop=(i == NMC - 1),
                                skip_group_check=True,
                            )
                        if fc == 1:
                            nc.tensor.matmul(
                                mel_ps[:], melnyq_t[:], prev_csq[0:1, :],
                                start=True, stop=False, skip_group_check=True,
                            )
                        if pend is not None:
                            pstft, pfc = pend
                            nc.tensor.matmul(
                                mel_ps[:], melT_t[:, pfc, :], pstft[:],
                                start=False, stop=False, skip_group_check=True,
                            )
                        ssq = sbe.tile([128, tt], dt.bfloat16, tag="ssq")
                        nc.scalar.activation(ssq[:], s_ps[:], AF.Square)
                        stft = sbe.tile([128, tt], dt.bfloat16, tag="stft")
                        nc.vector.tensor_tensor(
                            stft[:], csqs[fc][:], ssq[:], ALU.add
                        )
                        pend = (stft, fc)
                    last_pend = pend
                    continue
                for fc in range(NFC):
                    if fc == 4 and slot + 1 < len(slots):
                        # software-pipeline: fold the next slot's e/o now so
                        # the PE never waits on the DVE at slot boundaries
                        eo_next = emit_fold(slot + 1)
                    c_ps = psCS.tile([128, tt], dt.float32, tag="C")
                    s_ps = psCS.tile([128, tt], dt.float32, tag="S")
                    for i, mc in enumerate(MC_ORDER):
                        nc.tensor.matmul(
                            c_ps[:], wc_t[fc][:, mc, :], e_t[:, mc],
                            start=(i == 0), stop=(i == NMC - 1),
                            skip_group_check=True,
                        )
                    for i, mc in enumerate(MC_ORDER):
                        nc.tensor.matmul(
                            s_ps[:], ws_t[fc][:, mc, :], o_t[:, mc],
                            start=(i == 0), stop=(i == NMC - 1),
                            skip_group_check=True,
                        )
                    if fc == 1:
                        # rank-1 repair of the Nyquist fold (see header);
                        # first write of mel_ps (start=True)
                        nc.tensor.matmul(
                            mel_ps[:], melnyq_t[:], prev_csq[0:1, :],
                            start=True, stop=False, skip_group_check=True,
                        )
                    if pend is not None:
                        pstft, pfc = pend
                        nc.tensor.matmul(
                            mel_ps[:], melT_t[:, pfc, :], pstft[:],
                            start=False, stop=False, skip_group_check=True,
                        )
                    csq = sbe.tile([128, tt], dt.bfloat16, tag="csq")
                    ssq = sbe.tile([128, tt], dt.bfloat16, tag="ssq")
                    nc.scalar.activation(csq[:], c_ps[:], AF.Square)
                    nc.scalar.activation(ssq[:], s_ps[:], AF.Square)
                    if fc == 0:
                        prev_csq = csq
                    stft = sbe.tile([128, tt], dt.bfloat16, tag="stft")
                    nc.vector.tensor_tensor(stft[:], csq[:], ssq[:], ALU.add)
                    pend = (stft, fc)
                    if fc == 1 and slot > 0:
                        # previous slot's last mel matmul + epilogue, emitted
                        # here so its Square/add chain hides under this
                        # slot's DFT matmuls
                        lstft, lfc = last_pend
                        nc.tensor.matmul(
                            mel_pss[slot - 1][:], melT_t[:, lfc, :], lstft[:],
                            start=False, stop=True, skip_group_check=True,
                        )
                        emit_epilogue(slot - 1)
                last_pend = pend

            # last slot: flush the final mel matmul + epilogue directly
            lstft, lfc = last_pend
            nc.tensor.matmul(
                mel_pss[len(slots) - 1][:], melT_t[:, lfc, :], lstft[:],
                start=False, stop=True, skip_group_check=True,
            )
            # ---- local threshold, then AllReduce(max) of the threshold ----
            # The dB transform is monotone increasing, so
            # max_c f(lmax_c) == f(max_c lmax_c): compute the local o_thr
            # BEFORE the collective, and emit the whole chain ahead of the
            # last slot's Ln/affine so the collective triggers ASAP.
            cc_in = dram.tile([1, 128], dt.float32, name="cc_in")
            cc_out = dram.tile([1, 128], dt.float32, name="cc_out")

            def emit_thr_chain():
                lmax = sbw.tile([128, 1], dt.float32, name="lmax")
                nc.vector.tensor_reduce(
                    lmax[:], maxslots[:], mybir.AxisListType.X, ALU.max
                )
                gmax = sbw.tile([128, 1], dt.float32, name="gmax")
                nc.gpsimd.partition_all_reduce(
                    gmax[:], lmax[:], channels=128, reduce_op=bass_isa.ReduceOp.max
                )
                # ln(gmax * 1e-8) in one activation (scale folds the mult)
                thrln = sbw.tile([128, 1], dt.float32, name="thrln")
                nc.scalar.activation(thrln[:], gmax[:], AF.Ln, scale=TOPDB_LIN)
                lthr = sbw.tile([128, 1], dt.float32, name="lthr")
                nc.vector.tensor_scalar(
                    lthr[:], thrln[:], C_LOG / 80.0, 25.0 / 80.0, ALU.mult, ALU.add
                )
                # on the gpsimd queue: the collective trigger is also on
                # gpsimd, so no cross-engine semaphore handoff before it
                nc.gpsimd.dma_start(cc_in[:], lthr[:])
                nc.gpsimd.collective_compute(
                    "AllReduce",
                    ALU.max,
                    replica_groups=[list(range(NCORES))],
                    ins=[cc_in[:].opt()],
                    outs=[cc_out[:].opt()],
                )

            emit_epilogue(len(slots) - 1, defer=emit_thr_chain)
            o_thr = sbw.tile([128, 1], dt.float32, name="o_thr")
            nc.sync.dma_start(o_thr[:], cc_out[:])

            # ---- fixup: out = max(out_pre, o_thr), in-place, then DMA out ----
            qs = [nc.sync, nc.scalar]
            i = 0
            for b in range(BPC):
                for (t0, tt), od in zip(FIX_TILES, (out1_d, out2_d)):
                    oc = sbf.tile([128, tt], dt.bfloat16, tag="oc")
                    nc.vector.tensor_scalar(
                        oc[:], outp[b][:, t0 : t0 + tt], o_thr[:], None, ALU.max
                    )
                    qs[i % 2].dma_start(od.ap()[b], oc[:])
                    i += 1

    nc.compile()
    return nc


def _get_nc():
    if "nc" not in _compiled:
        _compiled["nc"] = _build_nc()
    return _compiled["nc"]


def _prep_inputs(x, cos_w, sin_w, mel_w):
    x = np.asarray(x, dtype=np.float32).reshape(B, T)
    wcf = np.asarray(cos_w, dtype=np.float32).reshape(WIN // 2 + 1, WIN)  # [1025,2048]
    wsf = np.asarray(sin_w, dtype=np.float32).reshape(WIN // 2 + 1, WIN)
    mel = np.asarray(mel_w, dtype=np.float32)  # [128, 1025]

    # x -> [B, 128, 2, 864]: C2[r, par, u] = x[256u + 128par + r], bf16,
    # plus the partition-reversed copy for the fold's mirrored operand.
    x16 = x.astype(ml_dtypes.bfloat16)
    c2 = np.ascontiguousarray(x16.reshape(B, UCOLS, 2, 128).transpose(0, 3, 2, 1))
    c2r = np.ascontiguousarray(c2[:, ::-1])

    # Folded weights from the provided arrays via the phase rotation:
    #   cos_w[f, 1024+m] = w~ cos(theta k),  sin_w[f, 1024+m] = -w~ sin(theta k)
    #   (k = 1024+m = j + 1023.5), phi_f = 2 pi f 1023.5 / 2048
    #   W~c[m,f] = w~ cos(theta j) = cos(phi) cos_w + sin(phi) (-sin_w)... computed below
    f = np.arange(WIN // 2 + 1, dtype=np.float64)
    phi = 2.0 * np.pi * f * 1023.5 / WIN
    cph = np.cos(phi)[:, None]
    sph = np.sin(phi)[:, None]
    A = wcf[:, 1024:].astype(np.float64)  # [1025, 1024] = w~ cos(theta k)
    Bp = wsf[:, 1024:].astype(np.float64)  # = -w~ sin(theta k)
    Wc = cph * A - sph * Bp  # [f, m] = w~ cos(theta j)
    Ws = -(cph * Bp + sph * A)  # = w~ sin(theta j)
    # S column for f=0 is exactly zero; carry the Nyquist S row there
    Ws[0] = Ws[1024]
    Wc_use = Wc[:1024]  # [1024 f, 1024 m]
    Ws_use = Ws[:1024]

    def dev_w(Wfm):  # [1024 f, 1024 m] -> [128 p, NFC, NMC, 128 fi]
        a = Wfm.reshape(NFC, 128, NMC, 128)  # [fc, fi, mc, p]
        return np.ascontiguousarray(a.transpose(3, 0, 2, 1)).astype(
            ml_dtypes.bfloat16
        )

    wc_dev = dev_w(Wc_use)
    ws_dev = dev_w(Ws_use)

    # mel column for f=0 becomes mel_w[:,1024] (applied to C_0^2 + S_nyq^2);
    # the rank-1 (mel_w[:,0]-mel_w[:,1024]) x C_0^2 term repairs it
    mel_mod = mel[:, :1024].copy()
    mel_mod[:, 0] = mel[:, 1024]
    melT = np.ascontiguousarray(
        mel_mod.T.reshape(NFC, 128, NMEL).transpose(1, 0, 2)
    ).astype(ml_dtypes.bfloat16)  # [128 fi, NFC, NMEL]
    melnyq = np.ascontiguousarray((mel[:, 0] - mel[:, 1024])[None, :]).astype(
        ml_dtypes.bfloat16
    )  # [1, NMEL]
    return c2, c2r, wc_dev, ws_dev, melT, melnyq


def _make_in_maps(inputs):
    c2, c2r, wc_dev, ws_dev, melT, melnyq = _prep_inputs(**inputs)
    in_maps = []
    for c in range(NCORES):
        in_maps.append(
            {
                "c2": c2[c * BPC : (c + 1) * BPC],
                "c2r": c2r[c * BPC : (c + 1) * BPC],
                "wc": wc_dev,
                "ws": ws_dev,
                "melT": melT,
                "melnyq": melnyq,
            }
        )
    return in_maps


def kernel(x, cos_w, sin_w, mel_w):
    nc = _get_nc()
    in_maps = _make_in_maps(
        {"x": x, "cos_w": cos_w, "sin_w": sin_w, "mel_w": mel_w}
    )
    res = run_bass_kernel_spmd(nc, in_maps, list(range(NCORES)))
    out = np.concatenate(
        [
            np.concatenate([r["out1"], r["out2"]], axis=2)
            for r in res.results
        ],
        axis=0,
    )  # [32,128,857]
    return out.astype(np.float32)


if __name__ == "__main__":
    rng = np.random.default_rng(0)
    x = rng.standard_normal((B, 1, T), dtype=np.float32)
    wc = rng.standard_normal((1025, 1, WIN), dtype=np.float32)
    wsn = rng.standard_normal((1025, 1, WIN), dtype=np.float32)
    mw = np.abs(rng.standard_normal((NMEL, 1025), dtype=np.float32)).astype(np.float32)
    o = kernel(x, wc, wsn, mw)
    print(o.shape, o.dtype)


# revision 30
# speedup vs baseline: 1.0859x; 1.0859x over previous
"""MelSpectrogramNet on 8 TRN2 NeuronCores (Bass/Tile), data-parallel over batch.

Math (per batch item):
  stft[f,t]  = (sum_k x[256t+k]*wc[f,k])^2 + (sum_k x[256t+k]*ws[f,k])^2
  mel        = mel_w @ stft
  x_db       = 10*log10(max(mel, 1e-10));  x_db = max(x_db, max_all(x_db)-80)
  out        = (x_db + 25) / 80

Folded DFT (the key trick): the hann window is exactly symmetric
(w[k] = w[2047-k]), so with j = k - 1023.5 the windowed DFT row is
w*cos(theta_f*j + phi_f). Folding x about the window center into
  e_m(t) = x[256t+1024+m] + x[256t+1023-m]
  o_m(t) = x[256t+1024+m] - x[256t+1023-m]        (m in [0,1024))
gives  cosDFT = cos(phi)C - sin(phi)S,  sinDFT = sin(phi)C + cos(phi)S with
  C_f = sum_m W~c[m,f] e_m,   S_f = sum_m W~s[m,f] o_m
and the power is phi-free:  power_f = C_f^2 + S_f^2.
=> the tensor-engine contraction halves (K=1024 per transform instead of
2x K=2048), which matters because the PE is the bottleneck (GPIO power
throttle caps it at 13/16 duty; the f32r version already ran at ~96% of
that throttled roofline).

Device mapping:
  - x is de-interleaved by 128-column parity into C2[r, par, u] =
    x[256u+128par+r] plus a partition-reversed copy C2R[r,...] =
    C2[127-r,...]; the DVE then computes each 128-row m-chunk of e/o as a
    single tensor_tensor add/sub of two contiguous slices (hidden under
    the matmuls of the previous tile).
  - all matmul operands are bf16 (measured end-to-end rel err ~5e-3 vs
    the 2e-2 gate); PSUM accumulation is fp32.
  - Nyquist: C_1024 = 0 exactly and the S-weight column for f=0 is exactly
    zero, so the S weights carry w~*(-1)^m (the Nyquist sine row) in the
    f=0 slot. Then stft[0] = C_0^2 + S_nyq^2; the mel weight column for
    f=0 is swapped to mel_w[:,1024] and a K=1 rank-1 matmul with
    (mel_w[:,0]-mel_w[:,1024]) x C_0^2 repairs the difference.
  - top_db clamp in linear space: pass 1 keeps out_pre in SBUF and the
    per-core max of mel; after gpsimd partition_all_reduce +
    AllReduce(max), the fixup applies out = max(out_pre, o_thr) in-place
    and DMAs straight out — no DRAM round-trip in the tail.
"""
import sys

sys.path.insert(0, "/opt/trn_rl_repo")

import ml_dtypes
import numpy as np

from concourse import bacc, bass_isa, mybir, tile
from concourse.bass_utils import run_bass_kernel_spmd

dt = mybir.dt
AF = mybir.ActivationFunctionType
ALU = mybir.AluOpType

NCORES = 8
B, T = 32, 221184
WIN, HOP = 2048, 256
FRAMES = (T - WIN) // HOP + 1  # 857
NMEL = 128
BPC = B // NCORES  # 4
UCOLS = T // 256  # 864 columns of 128 per parity
NFC = 8  # f-chunks of 128 (f = 0..1023); f=1024 (Nyquist) folded into S f=0
NMC = 8  # m-chunks of 128 (folded window half, m = 0..1023)
# Second tile overlaps the first by 3 frames so its width is a multiple of 4;
# overlapped frames are recomputed with identical values.
T_TILES = [(0, 512), (FRAMES - 348, 348)]
FIX_TILES = [(0, 512), (512, FRAMES - 512)]  # non-overlapping, for the fixup
C_LOG = 10.0 / float(np.log(10.0))  # 10*log10(x) = C_LOG * ln(x)
AMIN = 1e-10
TOPDB_LIN = 1e-8  # 10**(-80/10)

_compiled = {}


def _build_nc():
    nc = bacc.Bacc(
        "TRN2", target_bir_lowering=False, debug=False, num_devices=NCORES
    )

    c2_d = nc.dram_tensor("c2", [BPC, 128, 2, UCOLS], dt.bfloat16, kind="ExternalInput")
    c2r_d = nc.dram_tensor(
        "c2r", [BPC, 128, 2, UCOLS], dt.bfloat16, kind="ExternalInput"
    )
    wc_d = nc.dram_tensor("wc", [128, NFC, NMC, 128], dt.bfloat16, kind="ExternalInput")
    ws_d = nc.dram_tensor("ws", [128, NFC, NMC, 128], dt.bfloat16, kind="ExternalInput")
    melT_d = nc.dram_tensor("melT", [128, NFC, NMEL], dt.bfloat16, kind="ExternalInput")
    melnyq_d = nc.dram_tensor("melnyq", [1, NMEL], dt.bfloat16, kind="ExternalInput")
    out1_d = nc.dram_tensor("out1", [BPC, NMEL, 512], dt.bfloat16, kind="ExternalOutput")
    out2_d = nc.dram_tensor(
        "out2", [BPC, NMEL, FRAMES - 512], dt.bfloat16, kind="ExternalOutput"
    )

    with tile.TileContext(nc) as tc:
        with (
            tc.tile_pool(name="sbw", bufs=1) as sbw,
            tc.tile_pool(name="sbeo", bufs=2) as sbeo,
            tc.tile_pool(name="sbe", bufs=3) as sbe,
            tc.tile_pool(name="sbf", bufs=8) as sbf,
            tc.tile_pool(name="psCS", bufs=3, space="PSUM") as psCS,
            tc.tile_pool(name="psM", bufs=2, space="PSUM") as psM,
            tc.tile_pool(name="dram", bufs=1, space="DRAM") as dram,
        ):
            # persistent SBUF tensors
            c2s, c2rs, outp = [], [], []
            for b in range(BPC):
                c2s.append(sbw.tile([128, 2, UCOLS], dt.bfloat16, name=f"c2_{b}"))
                c2rs.append(sbw.tile([128, 2, UCOLS], dt.bfloat16, name=f"c2r_{b}"))
                outp.append(sbw.tile([128, FRAMES], dt.float32, name=f"outp_{b}"))
            wc_t = [sbw.tile([128, NMC, 128], dt.bfloat16, name=f"wc{fc}") for fc in range(NFC)]
            ws_t = [sbw.tile([128, NMC, 128], dt.bfloat16, name=f"ws{fc}") for fc in range(NFC)]
            melT_t = sbw.tile([128, NFC, NMEL], dt.bfloat16, name="melT_t")
            melnyq_t = sbw.tile([1, NMEL], dt.bfloat16, name="melnyq_t")
            nslots = BPC * len(T_TILES)
            maxslots = sbw.tile([128, nslots], dt.float32, name="maxslots")


            # ---- input DMAs: b=0 slices needed by the first tile go first.
            # fold mc=0 needs c2 parity 0 + c2r parity 1, so those two land
            # first.
            nc.gpsimd.dma_start(c2s[0][:, 0, 0:520], c2_d.ap()[0][:, 0, 0:520])
            nc.sync.dma_start(c2rs[0][:, 1, 0:520], c2r_d.ap()[0][:, 1, 0:520])
            nc.gpsimd.dma_start(c2s[0][:, 1, 0:520], c2_d.ap()[0][:, 1, 0:520])
            nc.gpsimd.dma_start(c2rs[0][:, 0, 0:520], c2r_d.ap()[0][:, 0, 0:520])
            # fc=0/1 weights split across sync/scalar queues so the first
            # matmuls are never DMA-starved; melT's first chunks land early
            # (the mel matmul is on the in-order PE queue — starving it
            # stalls the PE), then the remaining f-chunks alternate queues.
            nc.sync.dma_start(wc_t[0][:, 0:4], wc_d.ap()[:, 0, 0:4])
            nc.scalar.dma_start(wc_t[0][:, 4:], wc_d.ap()[:, 0, 4:])
            nc.sync.dma_start(ws_t[0][:, 0:4], ws_d.ap()[:, 0, 0:4])
            nc.scalar.dma_start(ws_t[0][:, 4:], ws_d.ap()[:, 0, 4:])
            nc.sync.dma_start(melnyq_t[:], melnyq_d.ap())
            nc.sync.dma_start(melT_t[:, 0:2], melT_d.ap()[:, 0:2])
            nc.sync.dma_start(wc_t[1][:], wc_d.ap()[:, 1])
            nc.scalar.dma_start(ws_t[1][:], ws_d.ap()[:, 1])

            # Warm up the collective engine while the DFT runs so the real
            # AllReduce at the end starts with rings already configured.
            ccw_in = dram.tile([1, 128], dt.float32, name="ccw_in")
            ccw_out = dram.tile([1, 128], dt.float32, name="ccw_out")
            nc.gpsimd.collective_compute(
                "AllReduce",
                ALU.max,
                replica_groups=[list(range(NCORES))],
                ins=[ccw_in[:].opt()],
                outs=[ccw_out[:].opt()],
            )

            nc.sync.dma_start(wc_t[2][:], wc_d.ap()[:, 2])
            nc.scalar.dma_start(ws_t[2][:], ws_d.ap()[:, 2])
            nc.sync.dma_start(melT_t[:, 2:], melT_d.ap()[:, 2:])
            for fc in range(3, NFC):
                nc.sync.dma_start(wc_t[fc][:], wc_d.ap()[:, fc])
                nc.scalar.dma_start(ws_t[fc][:], ws_d.ap()[:, fc])
            nc.gpsimd.dma_start(c2s[0][:, :, 520:], c2_d.ap()[0][:, :, 520:])
            nc.gpsimd.dma_start(c2rs[0][:, :, 520:], c2r_d.ap()[0][:, :, 520:])
            for b in range(1, BPC):
                nc.gpsimd.dma_start(c2s[b][:], c2_d.ap()[b])
                nc.gpsimd.dma_start(c2rs[b][:], c2r_d.ap()[b])

            # ---- pass 1: fold + folded DFT power + mel + log/affine ----
            slots = [(b, t0, tt) for b in range(BPC) for t0, tt in T_TILES]
            # even m-chunks need (c2 par0, c2r par1); odds the other pair —
            # process evens first so the first matmuls match DMA arrival order
            MC_ORDER = [0, 2, 4, 6, 1, 3, 5, 7]

            def emit_fold(si):
                # DVE fold: e/o m-chunks as adds/subs of shifted slices.
                # Slot 0 folds in 256-wide halves so work can start as soon
                # as the first x quarters land (keeps early PE gaps under
                # the ~3.4us HAM re-throttle window).
                b, t0, tt = slots[si]
                e_t = sbeo.tile([128, NMC, tt], dt.bfloat16, tag="e")
                o_t = sbeo.tile([128, NMC, tt], dt.bfloat16, tag="o")
                for mc in MC_ORDER:
                    p1 = mc % 2
                    u1 = t0 + 4 + mc // 2
                    p2 = 1 - p1
                    u2 = t0 + 3 - mc // 2
                    a = c2s[b][:, p1, u1 : u1 + tt]
                    r = c2rs[b][:, p2, u2 : u2 + tt]
                    nc.vector.tensor_tensor(e_t[:, mc], a, r, ALU.add)
                    nc.vector.tensor_tensor(o_t[:, mc], a, r, ALU.subtract)
                return e_t, o_t

            def emit_epilogue(slot, defer=None):
                # mel -> per-slot max -> clamp(AMIN) -> ln -> affine -> outp
                b, t0, tt = slots[slot]
                mel_ps = mel_pss[slot]
                mel_sb = sbe.tile([128, tt], dt.float32, tag="melsb")
                nc.vector.tensor_reduce(
                    maxslots[:, slot : slot + 1], mel_ps[:],
                    mybir.AxisListType.X, ALU.max,
                )
                if defer is not None:
                    defer()  # last slot: thr chain ahead of the Ln/affine
                nc.vector.tensor_scalar(mel_sb[:], mel_ps[:], AMIN, None, ALU.max)
                nc.scalar.activation(mel_sb[:], mel_sb[:], AF.Ln)
                nc.vector.tensor_scalar(
                    outp[b][:, t0 : t0 + tt], mel_sb[:],
                    C_LOG / 80.0, 25.0 / 80.0, ALU.mult, ALU.add,
                )

            eo_next = emit_fold(0)
            mel_pss = {}
            for slot, (b, t0, tt) in enumerate(slots):
                e_t, o_t = eo_next
                mel_ps = psM.tile([128, tt], dt.float32, tag="mel")
                mel_pss[slot] = mel_ps
                # mel matmuls are emitted one fc-iteration late so the
                # in-order PE queue never waits on the Square/add chain;
                # (stft tile, fc) pending between iterations:
                pend = None
                for fc in range(NFC):
                    if fc == 4 and slot + 1 < len(slots):
                        # software-pipeline: fold the next slot's e/o now so
                        # the PE never waits on the DVE at slot boundaries
                        eo_next = emit_fold(slot + 1)
                    c_ps = psCS.tile([128, tt], dt.float32, tag="C")
                    s_ps = psCS.tile([128, tt], dt.float32, tag="S")
                    for i, mc in enumerate(MC_ORDER):
                        nc.tensor.matmul(
                            c_ps[:], wc_t[fc][:, mc, :], e_t[:, mc],
                            start=(i == 0), stop=(i == NMC - 1),
                            skip_group_check=True,
                        )
                    for i, mc in enumerate(MC_ORDER):
                        nc.tensor.matmul(
                            s_ps[:], ws_t[fc][:, mc, :], o_t[:, mc],
                            start=(i == 0), stop=(i == NMC - 1),
                            skip_group_check=True,
                        )
                    if fc == 1:
                        # rank-1 repair of the Nyquist fold (see header);
                        # first write of mel_ps (start=True)
                        nc.tensor.matmul(
                            mel_ps[:], melnyq_t[:], prev_csq[0:1, :],
                            start=True, stop=False, skip_group_check=True,
                        )
                    if pend is not None:
                        pstft, pfc = pend
                        nc.tensor.matmul(
                            mel_ps[:], melT_t[:, pfc, :], pstft[:],
                            start=False, stop=False, skip_group_check=True,
                        )
                    csq = sbe.tile([128, tt], dt.bfloat16, tag="csq")
                    ssq = sbe.tile([128, tt], dt.bfloat16, tag="ssq")
                    nc.scalar.activation(csq[:], c_ps[:], AF.Square)
                    nc.scalar.activation(ssq[:], s_ps[:], AF.Square)
                    if fc == 0:
                        prev_csq = csq
                    stft = sbe.tile([128, tt], dt.bfloat16, tag="stft")
                    nc.vector.tensor_tensor(stft[:], csq[:], ssq[:], ALU.add)
                    pend = (stft, fc)
                    if fc == 1 and slot > 0:
                        # previous slot's last mel matmul + epilogue, emitted
                        # here so its Square/add chain hides under this
                        # slot's DFT matmuls
                        lstft, lfc = last_pend
                        nc.tensor.matmul(
                            mel_pss[slot - 1][:], melT_t[:, lfc, :], lstft[:],
                            start=False, stop=True, skip_group_check=True,
                        )
                        emit_epilogue(slot - 1)
                last_pend = pend

            # last slot: flush the final mel matmul + epilogue directly
            lstft, lfc = last_pend
            nc.tensor.matmul(
                mel_pss[len(slots) - 1][:], melT_t[:, lfc, :], lstft[:],
                start=False, stop=True, skip_group_check=True,
            )
            # ---- local threshold, then AllReduce(max) of the threshold ----
            # The dB transform is monotone increasing, so
            # max_c f(lmax_c) == f(max_c lmax_c): compute the local o_thr
            # BEFORE the collective, and emit the whole chain ahead of the
            # last slot's Ln/affine so the collective triggers ASAP.
            cc_in = dram.tile([1, 128], dt.float32, name="cc_in")
            cc_out = dram.tile([1, 128], dt.float32, name="cc_out")

            def emit_thr_chain():
                lmax = sbw.tile([128, 1], dt.float32, name="lmax")
                nc.vector.tensor_reduce(
                    lmax[:], maxslots[:], mybir.AxisListType.X, ALU.max
                )
                gmax = sbw.tile([128, 1], dt.float32, name="gmax")
                nc.gpsimd.partition_all_reduce(
                    gmax[:], lmax[:], channels=128, reduce_op=bass_isa.ReduceOp.max
                )
                # ln(gmax * 1e-8) in one activation (scale folds the mult)
                thrln = sbw.tile([128, 1], dt.float32, name="thrln")
                nc.scalar.activation(thrln[:], gmax[:], AF.Ln, scale=TOPDB_LIN)
                lthr = sbw.tile([128, 1], dt.float32, name="lthr")
                nc.vector.tensor_scalar(
                    lthr[:], thrln[:], C_LOG / 80.0, 25.0 / 80.0, ALU.mult, ALU.add
                )
                # on the gpsimd queue: the collective trigger is also on
                # gpsimd, so no cross-engine semaphore handoff before it
                nc.gpsimd.dma_start(cc_in[:], lthr[:])
                nc.gpsimd.collective_compute(
                    "AllReduce",
                    ALU.max,
                    replica_groups=[list(range(NCORES))],
                    ins=[cc_in[:].opt()],
                    outs=[cc_out[:].opt()],
                )

            emit_epilogue(len(slots) - 1, defer=emit_thr_chain)
            o_thr = sbw.tile([128, 1], dt.float32, name="o_thr")
            nc.sync.dma_start(o_thr[:], cc_out[:])

            # ---- fixup: out = max(out_pre, o_thr), in-place, then DMA out ----
            qs = [nc.sync, nc.scalar]
            i = 0
            for b in range(BPC):
                for (t0, tt), od in zip(FIX_TILES, (out1_d, out2_d)):
                    oc = sbf.tile([128, tt], dt.bfloat16, tag="oc")
                    nc.vector.tensor_scalar(
                        oc[:], outp[b][:, t0 : t0 + tt], o_thr[:], None, ALU.max
                    )
                    qs[i % 2].dma_start(od.ap()[b], oc[:])
                    i += 1

    nc.compile()
    return nc


def _get_nc():
    if "nc" not in _compiled:
        _compiled["nc"] = _build_nc()
    return _compiled["nc"]


def _prep_inputs(x, cos_w, sin_w, mel_w):
    x = np.asarray(x, dtype=np.float32).reshape(B, T)
    wcf = np.asarray(cos_w, dtype=np.float32).reshape(WIN // 2 + 1, WIN)  # [1025,2048]
    wsf = np.asarray(sin_w, dtype=np.float32).reshape(WIN // 2 + 1, WIN)
    mel = np.asarray(mel_w, dtype=np.float32)  # [128, 1025]

    # x -> [B, 128, 2, 864]: C2[r, par, u] = x[256u + 128par + r], bf16,
    # plus the partition-reversed copy for the fold's mirrored operand.
    x16 = x.astype(ml_dtypes.bfloat16)
    c2 = np.ascontiguousarray(x16.reshape(B, UCOLS, 2, 128).transpose(0, 3, 2, 1))
    c2r = np.ascontiguousarray(c2[:, ::-1])

    # Folded weights from the provided arrays via the phase rotation:
    #   cos_w[f, 1024+m] = w~ cos(theta k),  sin_w[f, 1024+m] = -w~ sin(theta k)
    #   (k = 1024+m = j + 1023.5), phi_f = 2 pi f 1023.5 / 2048
    #   W~c[m,f] = w~ cos(theta j) = cos(phi) cos_w + sin(phi) (-sin_w)... computed below
    f = np.arange(WIN // 2 + 1, dtype=np.float64)
    phi = 2.0 * np.pi * f * 1023.5 / WIN
    cph = np.cos(phi)[:, None]
    sph = np.sin(phi)[:, None]
    A = wcf[:, 1024:].astype(np.float64)  # [1025, 1024] = w~ cos(theta k)
    Bp = wsf[:, 1024:].astype(np.float64)  # = -w~ sin(theta k)
    Wc = cph * A - sph * Bp  # [f, m] = w~ cos(theta j)
    Ws = -(cph * Bp + sph * A)  # = w~ sin(theta j)
    # S column for f=0 is exactly zero; carry the Nyquist S row there
    Ws[0] = Ws[1024]
    Wc_use = Wc[:1024]  # [1024 f, 1024 m]
    Ws_use = Ws[:1024]

    def dev_w(Wfm):  # [1024 f, 1024 m] -> [128 p, NFC, NMC, 128 fi]
        a = Wfm.reshape(NFC, 128, NMC, 128)  # [fc, fi, mc, p]
        return np.ascontiguousarray(a.transpose(3, 0, 2, 1)).astype(
            ml_dtypes.bfloat16
        )

    wc_dev = dev_w(Wc_use)
    ws_dev = dev_w(Ws_use)

    # mel column for f=0 becomes mel_w[:,1024] (applied to C_0^2 + S_nyq^2);
    # the rank-1 (mel_w[:,0]-mel_w[:,1024]) x C_0^2 term repairs it
    mel_mod = mel[:, :1024].copy()
    mel_mod[:, 0] = mel[:, 1024]
    melT = np.ascontiguousarray(
        mel_mod.T.reshape(NFC, 128, NMEL).transpose(1, 0, 2)
    ).astype(ml_dtypes.bfloat16)  # [128 fi, NFC, NMEL]
    melnyq = np.ascontiguousarray((mel[:, 0] - mel[:, 1024])[None, :]).astype(
        ml_dtypes.bfloat16
    )  # [1, NMEL]
    return c2, c2r, wc_dev, ws_dev, melT, melnyq


def _make_in_maps(inputs):
    c2, c2r, wc_dev, ws_dev, melT, melnyq = _prep_inputs(**inputs)
    in_maps = []
    for c in range(NCORES):
        in_maps.append(
            {
                "c2": c2[c * BPC : (c + 1) * BPC],
                "c2r": c2r[c * BPC : (c + 1) * BPC],
                "wc": wc_dev,
                "ws": ws_dev,
                "melT": melT,
                "melnyq": melnyq,
            }
        )
    return in_maps


def kernel(x, cos_w, sin_w, mel_w):
    nc = _get_nc()
    in_maps = _make_in_maps(
        {"x": x, "cos_w": cos_w, "sin_w": sin_w, "mel_w": mel_w}
    )
    res = run_bass_kernel_spmd(nc, in_maps, list(range(NCORES)))
    out = np.concatenate(
        [
            np.concatenate([r["out1"], r["out2"]], axis=2)
            for r in res.results
        ],
        axis=0,
    )  # [32,128,857]
    return out.astype(np.float32)


if __name__ == "__main__":
    rng = np.random.default_rng(0)
    x = rng.standard_normal((B, 1, T), dtype=np.float32)
    wc = rng.standard_normal((1025, 1, WIN), dtype=np.float32)
    wsn = rng.standard_normal((1025, 1, WIN), dtype=np.float32)
    mw = np.abs(rng.standard_normal((NMEL, 1025), dtype=np.float32)).astype(np.float32)
    o = kernel(x, wc, wsn, mw)
    print(o.shape, o.dtype)


# revision 36
# speedup vs baseline: 1.4226x; 1.3101x over previous
"""MelSpectrogramNet on 8 TRN2 NeuronCores (Bass/Tile), data-parallel over batch.

Math (per batch item):
  stft[f,t]  = (sum_k x[256t+k]*wc[f,k])^2 + (sum_k x[256t+k]*ws[f,k])^2
  mel        = mel_w @ stft
  x_db       = 10*log10(max(mel, 1e-10));  x_db = max(x_db, max_all(x_db)-80)
  out        = (x_db + 25) / 80

Folded DFT (the key trick): the hann window is exactly symmetric
(w[k] = w[2047-k]), so with j = k - 1023.5 the windowed DFT row is
w*cos(theta_f*j + phi_f). Folding x about the window center into
  e_m(t) = x[256t+1024+m] + x[256t+1023-m]
  o_m(t) = x[256t+1024+m] - x[256t+1023-m]        (m in [0,1024))
gives  cosDFT = cos(phi)C - sin(phi)S,  sinDFT = sin(phi)C + cos(phi)S with
  C_f = sum_m W~c[m,f] e_m,   S_f = sum_m W~s[m,f] o_m
and the power is phi-free:  power_f = C_f^2 + S_f^2.
=> the tensor-engine contraction halves (K=1024 per transform instead of
2x K=2048), which matters because the PE is the bottleneck (GPIO power
throttle caps it at 13/16 duty; the f32r version already ran at ~96% of
that throttled roofline).

Device mapping:
  - x is de-interleaved by 128-column parity into C2[r, par, u] =
    x[256u+128par+r] plus a partition-reversed copy C2R[r,...] =
    C2[127-r,...]; the DVE then computes each 128-row m-chunk of e/o as a
    single tensor_tensor add/sub of two contiguous slices (hidden under
    the matmuls of the previous tile).
  - all matmul operands are bf16 (measured end-to-end rel err ~5e-3 vs
    the 2e-2 gate); PSUM accumulation is fp32.
  - Nyquist: C_1024 = 0 exactly and the S-weight column for f=0 is exactly
    zero, so the S weights carry w~*(-1)^m (the Nyquist sine row) in the
    f=0 slot. Then stft[0] = C_0^2 + S_nyq^2; the mel weight column for
    f=0 is swapped to mel_w[:,1024] and a K=1 rank-1 matmul with
    (mel_w[:,0]-mel_w[:,1024]) x C_0^2 repairs the difference.
  - top_db clamp in linear space: pass 1 keeps out_pre in SBUF and the
    per-core max of mel; after gpsimd partition_all_reduce +
    AllReduce(max), the fixup applies out = max(out_pre, o_thr) in-place
    and DMAs straight out — no DRAM round-trip in the tail.
"""
import sys

sys.path.insert(0, "/opt/trn_rl_repo")

import ml_dtypes
import numpy as np

from concourse import bacc, bass_isa, mybir, tile
from concourse.bass_utils import run_bass_kernel_spmd

dt = mybir.dt
AF = mybir.ActivationFunctionType
ALU = mybir.AluOpType

NCORES = 8
B, T = 32, 221184
WIN, HOP = 2048, 256
FRAMES = (T - WIN) // HOP + 1  # 857
NMEL = 128
BPC = B // NCORES  # 4
UCOLS = T // 256  # 864 columns of 128 per parity
NFC = 8  # f-chunks of 128 (f = 0..1023); f=1024 (Nyquist) folded into S f=0
NMC = 8  # m-chunks of 128 (folded window half, m = 0..1023)
NMC2 = 4  # m-chunks after the SECOND fold (m = 0..511)
# Second tile overlaps the first by 3 frames so its width is a multiple of 4;
# overlapped frames are recomputed with identical values.
T_TILES = [(0, 512), (FRAMES - 348, 348)]
FIX_TILES = [(0, 512), (512, FRAMES - 512)]  # non-overlapping, for the fixup
C_LOG = 10.0 / float(np.log(10.0))  # 10*log10(x) = C_LOG * ln(x)
AMIN = 1e-10
TOPDB_LIN = 1e-8  # 10**(-80/10)

_compiled = {}


def _build_nc():
    nc = bacc.Bacc(
        "TRN2", target_bir_lowering=False, debug=False, num_devices=NCORES
    )

    c2_d = nc.dram_tensor("c2", [BPC, 128, 2, UCOLS], dt.bfloat16, kind="ExternalInput")
    c2r_d = nc.dram_tensor(
        "c2r", [BPC, 128, 2, UCOLS], dt.bfloat16, kind="ExternalInput"
    )
    wc_d = nc.dram_tensor("wc", [128, NFC, NMC2, 128], dt.bfloat16, kind="ExternalInput")
    ws_d = nc.dram_tensor("ws", [128, NFC, NMC2, 128], dt.bfloat16, kind="ExternalInput")
    wv_d = nc.dram_tensor("wv", [128, NMC], dt.float32, kind="ExternalInput")
    wrv_d = nc.dram_tensor("wrv", [128, NMC], dt.float32, kind="ExternalInput")
    melT_d = nc.dram_tensor("melT", [128, NFC, NMEL], dt.bfloat16, kind="ExternalInput")
    melnyq_d = nc.dram_tensor("melnyq", [1, NMEL], dt.bfloat16, kind="ExternalInput")
    out1_d = nc.dram_tensor("out1", [BPC, NMEL, 512], dt.bfloat16, kind="ExternalOutput")
    out2_d = nc.dram_tensor(
        "out2", [BPC, NMEL, FRAMES - 512], dt.bfloat16, kind="ExternalOutput"
    )

    with tile.TileContext(nc) as tc:
        with (
            tc.tile_pool(name="sbw", bufs=1) as sbw,
            tc.tile_pool(name="sbeo", bufs=2) as sbeo,
            tc.tile_pool(name="sbe", bufs=3) as sbe,
            tc.tile_pool(name="sbf", bufs=8) as sbf,
            tc.tile_pool(name="sbt", bufs=2) as sbt,
            tc.tile_pool(name="psCS", bufs=3, space="PSUM") as psCS,
            tc.tile_pool(name="psM", bufs=2, space="PSUM") as psM,
            tc.tile_pool(name="dram", bufs=1, space="DRAM") as dram,
        ):
            # persistent SBUF tensors
            c2s, c2rs, outp = [], [], []
            for b in range(BPC):
                c2s.append(sbw.tile([128, 2, UCOLS], dt.bfloat16, name=f"c2_{b}"))
                c2rs.append(sbw.tile([128, 2, UCOLS], dt.bfloat16, name=f"c2r_{b}"))
                outp.append(sbw.tile([128, FRAMES], dt.float32, name=f"outp_{b}"))
            wc_t = [sbw.tile([128, NMC2, 128], dt.bfloat16, name=f"wc{fc}") for fc in range(NFC)]
            ws_t = [sbw.tile([128, NMC2, 128], dt.bfloat16, name=f"ws{fc}") for fc in range(NFC)]
            wv_t = sbw.tile([128, NMC], dt.float32, name="wv_t")
            wrv_t = sbw.tile([128, NMC], dt.float32, name="wrv_t")
            melT_t = sbw.tile([128, NFC, NMEL], dt.bfloat16, name="melT_t")
            melnyq_t = sbw.tile([1, NMEL], dt.bfloat16, name="melnyq_t")
            nslots = BPC * len(T_TILES)
            maxslots = sbw.tile([128, nslots], dt.float32, name="maxslots")


            # ---- input DMAs: b=0 slices needed by the first tile go first.
            # fold mc=0 needs c2 parity 0 + c2r parity 1, so those two land
            # first.
            nc.gpsimd.dma_start(c2s[0][:, 0, 0:520], c2_d.ap()[0][:, 0, 0:520])
            nc.sync.dma_start(c2rs[0][:, 1, 0:520], c2r_d.ap()[0][:, 1, 0:520])
            nc.gpsimd.dma_start(c2s[0][:, 1, 0:520], c2_d.ap()[0][:, 1, 0:520])
            nc.gpsimd.dma_start(c2rs[0][:, 0, 0:520], c2r_d.ap()[0][:, 0, 0:520])
            # fc=0/1 weights split across sync/scalar queues so the first
            # matmuls are never DMA-starved; melT's first chunks land early
            # (the mel matmul is on the in-order PE queue — starving it
            # stalls the PE), then the remaining f-chunks alternate queues.
            nc.sync.dma_start(wv_t[:], wv_d.ap())
            nc.sync.dma_start(wrv_t[:], wrv_d.ap())
            nc.sync.dma_start(wc_t[0][:, 0:2], wc_d.ap()[:, 0, 0:2])
            nc.scalar.dma_start(wc_t[0][:, 2:], wc_d.ap()[:, 0, 2:])
            nc.sync.dma_start(ws_t[0][:, 0:2], ws_d.ap()[:, 0, 0:2])
            nc.scalar.dma_start(ws_t[0][:, 2:], ws_d.ap()[:, 0, 2:])
            nc.sync.dma_start(melnyq_t[:], melnyq_d.ap())
            nc.sync.dma_start(melT_t[:, 0:2], melT_d.ap()[:, 0:2])
            nc.sync.dma_start(wc_t[1][:], wc_d.ap()[:, 1])
            nc.scalar.dma_start(ws_t[1][:], ws_d.ap()[:, 1])

            # Warm up the collective engine while the DFT runs so the real
            # AllReduce at the end starts with rings already configured.
            ccw_in = dram.tile([1, 128], dt.float32, name="ccw_in")
            ccw_out = dram.tile([1, 128], dt.float32, name="ccw_out")
            nc.gpsimd.collective_compute(
                "AllReduce",
                ALU.max,
                replica_groups=[list(range(NCORES))],
                ins=[ccw_in[:].opt()],
                outs=[ccw_out[:].opt()],
            )

            nc.sync.dma_start(wc_t[2][:], wc_d.ap()[:, 2])
            nc.scalar.dma_start(ws_t[2][:], ws_d.ap()[:, 2])
            nc.sync.dma_start(melT_t[:, 2:], melT_d.ap()[:, 2:])
            for fc in range(3, NFC):
                nc.sync.dma_start(wc_t[fc][:], wc_d.ap()[:, fc])
                nc.scalar.dma_start(ws_t[fc][:], ws_d.ap()[:, fc])
            nc.gpsimd.dma_start(c2s[0][:, :, 520:], c2_d.ap()[0][:, :, 520:])
            nc.gpsimd.dma_start(c2rs[0][:, :, 520:], c2r_d.ap()[0][:, :, 520:])
            for b in range(1, BPC):
                nc.gpsimd.dma_start(c2s[b][:], c2_d.ap()[b])
                nc.gpsimd.dma_start(c2rs[b][:], c2r_d.ap()[b])

            # ---- pass 1: fold + folded DFT power + mel + log/affine ----
            slots = [(b, t0, tt) for b in range(BPC) for t0, tt in T_TILES]
            # even m-chunks need (c2 par0, c2r par1); odds the other pair —
            # process evens first so the first matmuls match DMA arrival order
            MC_ORDER = [0, 2, 4, 6, 1, 3, 5, 7]

            def emit_fold(si):
                # DVE fold: e/o m-chunks as adds/subs of shifted slices.
                # Slot 0 folds in 256-wide halves so work can start as soon
                # as the first x quarters land (keeps early PE gaps under
                # the ~3.4us HAM re-throttle window).
                # DVE double-fold. First fold (window-center symmetry):
                #   e_m = x[256t+1024+m] + x[256t+1023-m]   (m in [0,1024))
                # then window ON THE DATA (ew = w~*e) and fold again about
                # m <-> 1023-m (cos kernel parity: the mirrored half enters
                # with sign (-1)^f):
                #   E+/-_m = ew_m +/- ew_{1023-m}           (m in [0,512))
                # and analogously for the sine side. Even frequencies use
                # (E+, O-), odd use (E-, O+): contraction K halves to 512.
                b, t0, tt = slots[si]
                EP = sbeo.tile([128, NMC2, tt], dt.bfloat16, tag="EP")
                EM = sbeo.tile([128, NMC2, tt], dt.bfloat16, tag="EM")
                OP = sbeo.tile([128, NMC2, tt], dt.bfloat16, tag="OP")
                OM = sbeo.tile([128, NMC2, tt], dt.bfloat16, tag="OM")
                for mc in range(NMC2):
                    # direct half (m = 128mc+p): x[256t+1024+m], x[256t+1023-m]
                    u1 = t0 + 4 + mc // 2
                    u2 = t0 + 3 - mc // 2
                    a = c2s[b][:, mc % 2, u1 : u1 + tt]
                    r = c2rs[b][:, 1 - mc % 2, u2 : u2 + tt]
                    # mirrored half (m~ = 1023-m): x[256t+2047-m], x[256t+m]
                    q = 15 - mc
                    u3 = t0 + q // 2
                    u4 = t0 + mc // 2
                    am = c2rs[b][:, q % 2, u3 : u3 + tt]
                    rm = c2s[b][:, mc % 2, u4 : u4 + tt]
                    ew = sbt.tile([128, tt], dt.bfloat16, tag="ew")
                    rw = sbt.tile([128, tt], dt.bfloat16, tag="rw")
                    nc.vector.tensor_tensor(ew[:], a, r, ALU.add)
                    nc.vector.tensor_scalar(
                        ew[:], ew[:], wv_t[:, mc : mc + 1], None, ALU.mult
                    )
                    nc.vector.tensor_tensor(rw[:], am, rm, ALU.add)
                    nc.vector.tensor_scalar(
                        rw[:], rw[:], wrv_t[:, mc : mc + 1], None, ALU.mult
                    )
                    nc.vector.tensor_tensor(EP[:, mc], ew[:], rw[:], ALU.add)
                    nc.vector.tensor_tensor(EM[:, mc], ew[:], rw[:], ALU.subtract)
                    ow = sbt.tile([128, tt], dt.bfloat16, tag="ow")
                    row = sbt.tile([128, tt], dt.bfloat16, tag="row")
                    nc.vector.tensor_tensor(ow[:], a, r, ALU.subtract)
                    nc.vector.tensor_scalar(
                        ow[:], ow[:], wv_t[:, mc : mc + 1], None, ALU.mult
                    )
                    nc.vector.tensor_tensor(row[:], am, rm, ALU.subtract)
                    nc.vector.tensor_scalar(
                        row[:], row[:], wrv_t[:, mc : mc + 1], None, ALU.mult
                    )
                    nc.vector.tensor_tensor(OP[:, mc], ow[:], row[:], ALU.add)
                    nc.vector.tensor_tensor(OM[:, mc], ow[:], row[:], ALU.subtract)
                return (EP, EM, OP, OM)

            def emit_epilogue(slot, defer=None):
                # mel -> per-slot max -> clamp(AMIN) -> ln -> affine -> outp
                b, t0, tt = slots[slot]
                mel_ps = mel_pss[slot]
                mel_sb = sbe.tile([128, tt], dt.float32, tag="melsb")
                nc.vector.tensor_reduce(
                    maxslots[:, slot : slot + 1], mel_ps[:],
                    mybir.AxisListType.X, ALU.max,
                )
                if defer is not None:
                    defer()  # last slot: thr chain ahead of the Ln/affine
                nc.vector.tensor_scalar(mel_sb[:], mel_ps[:], AMIN, None, ALU.max)
                nc.scalar.activation(mel_sb[:], mel_sb[:], AF.Ln)
                nc.vector.tensor_scalar(
                    outp[b][:, t0 : t0 + tt], mel_sb[:],
                    C_LOG / 80.0, 25.0 / 80.0, ALU.mult, ALU.add,
                )

            eo_next = emit_fold(0)
            mel_pss = {}
            for slot, (b, t0, tt) in enumerate(slots):
                EPc, EMc, OPc, OMc = eo_next
                mel_ps = psM.tile([128, tt], dt.float32, tag="mel")
                mel_pss[slot] = mel_ps
                # mel matmuls are emitted one fc-iteration late so the
                # in-order PE queue never waits on the Square/add chain;
                # (stft tile, fc) pending between iterations:
                pend = None
                for fc in range(NFC):
                    if fc == 4 and slot + 1 < len(slots):
                        # software-pipeline: fold the next slot's e/o now so
                        # the PE never waits on the DVE at slot boundaries
                        eo_next = emit_fold(slot + 1)
                    c_ps = psCS.tile([128, tt], dt.float32, tag="C")
                    s_ps = psCS.tile([128, tt], dt.float32, tag="S")
                    cmov = EPc if fc < 4 else EMc
                    smov = OMc if fc < 4 else OPc
                    for mc in range(NMC2):
                        nc.tensor.matmul(
                            c_ps[:], wc_t[fc][:, mc, :], cmov[:, mc],
                            start=(mc == 0), stop=(mc == NMC2 - 1),
                            skip_group_check=True,
                        )
                    for mc in range(NMC2):
                        nc.tensor.matmul(
                            s_ps[:], ws_t[fc][:, mc, :], smov[:, mc],
                            start=(mc == 0), stop=(mc == NMC2 - 1),
                            skip_group_check=True,
                        )
                    if fc == 1:
                        # rank-1 repair of the Nyquist fold (see header);
                        # first write of mel_ps (start=True)
                        nc.tensor.matmul(
                            mel_ps[:], melnyq_t[:], prev_csq[0:1, :],
                            start=True, stop=False, skip_group_check=True,
                        )
                    if pend is not None:
                        pstft, pfc = pend
                        nc.tensor.matmul(
                            mel_ps[:], melT_t[:, pfc, :], pstft[:],
                            start=False, stop=False, skip_group_check=True,
                        )
                    csq = sbe.tile([128, tt], dt.bfloat16, tag="csq")
                    ssq = sbe.tile([128, tt], dt.bfloat16, tag="ssq")
                    nc.scalar.activation(csq[:], c_ps[:], AF.Square)
                    nc.scalar.activation(ssq[:], s_ps[:], AF.Square)
                    if fc == 0:
                        prev_csq = csq
                    stft = sbe.tile([128, tt], dt.bfloat16, tag="stft")
                    nc.vector.tensor_tensor(stft[:], csq[:], ssq[:], ALU.add)
                    pend = (stft, fc)
                    if fc == 1 and slot > 0:
                        # previous slot's last mel matmul + epilogue, emitted
                        # here so its Square/add chain hides under this
                        # slot's DFT matmuls
                        lstft, lfc = last_pend
                        nc.tensor.matmul(
                            mel_pss[slot - 1][:], melT_t[:, lfc, :], lstft[:],
                            start=False, stop=True, skip_group_check=True,
                        )
                        emit_epilogue(slot - 1)
                last_pend = pend

            # last slot: flush the final mel matmul + epilogue directly
            lstft, lfc = last_pend
            nc.tensor.matmul(
                mel_pss[len(slots) - 1][:], melT_t[:, lfc, :], lstft[:],
                start=False, stop=True, skip_group_check=True,
            )
            # ---- local threshold, then AllReduce(max) of the threshold ----
            # The dB transform is monotone increasing, so
            # max_c f(lmax_c) == f(max_c lmax_c): compute the local o_thr
            # BEFORE the collective, and emit the whole chain ahead of the
            # last slot's Ln/affine so the collective triggers ASAP.
            cc_in = dram.tile([1, 128], dt.float32, name="cc_in")
            cc_out = dram.tile([1, 128], dt.float32, name="cc_out")

            def emit_thr_chain():
                lmax = sbw.tile([128, 1], dt.float32, name="lmax")
                nc.vector.tensor_reduce(
                    lmax[:], maxslots[:], mybir.AxisListType.X, ALU.max
                )
                gmax = sbw.tile([128, 1], dt.float32, name="gmax")
                nc.gpsimd.partition_all_reduce(
                    gmax[:], lmax[:], channels=128, reduce_op=bass_isa.ReduceOp.max
                )
                # ln(gmax * 1e-8) in one activation (scale folds the mult)
                thrln = sbw.tile([128, 1], dt.float32, name="thrln")
                nc.scalar.activation(thrln[:], gmax[:], AF.Ln, scale=TOPDB_LIN)
                lthr = sbw.tile([128, 1], dt.float32, name="lthr")
                nc.vector.tensor_scalar(
                    lthr[:], thrln[:], C_LOG / 80.0, 25.0 / 80.0, ALU.mult, ALU.add
                )
                # on the gpsimd queue: the collective trigger is also on
                # gpsimd, so no cross-engine semaphore handoff before it
                nc.gpsimd.dma_start(cc_in[:], lthr[:])
                nc.gpsimd.collective_compute(
                    "AllReduce",
                    ALU.max,
                    replica_groups=[list(range(NCORES))],
                    ins=[cc_in[:].opt()],
                    outs=[cc_out[:].opt()],
                )

            emit_epilogue(len(slots) - 1, defer=emit_thr_chain)
            o_thr = sbw.tile([128, 1], dt.float32, name="o_thr")
            nc.sync.dma_start(o_thr[:], cc_out[:])

            # ---- fixup: out = max(out_pre, o_thr), in-place, then DMA out ----
            qs = [nc.sync, nc.scalar]
            i = 0
            for b in range(BPC):
                for (t0, tt), od in zip(FIX_TILES, (out1_d, out2_d)):
                    oc = sbf.tile([128, tt], dt.bfloat16, tag="oc")
                    nc.vector.tensor_scalar(
                        oc[:], outp[b][:, t0 : t0 + tt], o_thr[:], None, ALU.max
                    )
                    qs[i % 2].dma_start(od.ap()[b], oc[:])
                    i += 1

    nc.compile()
    return nc


def _get_nc():
    if "nc" not in _compiled:
        _compiled["nc"] = _build_nc()
    return _compiled["nc"]


def _prep_inputs(x, cos_w, sin_w, mel_w):
    x = np.asarray(x, dtype=np.float32).reshape(B, T)
    wcf = np.asarray(cos_w, dtype=np.float32).reshape(WIN // 2 + 1, WIN)  # [1025,2048]
    wsf = np.asarray(sin_w, dtype=np.float32).reshape(WIN // 2 + 1, WIN)
    mel = np.asarray(mel_w, dtype=np.float32)  # [128, 1025]

    # x -> [B, 128, 2, 864]: C2[r, par, u] = x[256u + 128par + r], bf16,
    # plus the partition-reversed copy for the fold's mirrored operand.
    x16 = x.astype(ml_dtypes.bfloat16)
    c2 = np.ascontiguousarray(x16.reshape(B, UCOLS, 2, 128).transpose(0, 3, 2, 1))
    c2r = np.ascontiguousarray(c2[:, ::-1])

    # Double-fold prep. The window (applied on-chip to the folded data)
    # comes from the provided cos_w f=0 row, which is exactly hann:
    wfull = wcf[0, 1024:].astype(np.float64)  # [1024] = hann[1024:]
    wv = np.ascontiguousarray(
        wfull.reshape(NMC, 128).T
    ).astype(np.float32)  # wv[p, mc] = w~[128mc+p]
    wrv = np.ascontiguousarray(
        wfull[::-1].reshape(NMC, 128).T
    ).astype(np.float32)  # wrv[p, mc] = w~[1023-128mc-p]

    # Pure-trig second-fold weights; frequencies regrouped by parity:
    # chunks 0-3 = even f (0,2,..,1022), chunks 4-7 = odd f (1,3,..,1023).
    m2 = np.arange(512, dtype=np.float64)
    j2 = m2 + 0.5
    fs = np.concatenate([2 * np.arange(512), 2 * np.arange(512) + 1])
    th = 2.0 * np.pi * fs[None, :].astype(np.float64) / WIN
    Wc2 = np.cos(th * j2[:, None])  # [512 m2, 1024 arranged f]
    Ws2 = np.sin(th * j2[:, None])
    # arranged position 0 is f=0 whose S row is exactly zero; carry the
    # Nyquist S row there: sin(pi*(m2+0.5)) = (-1)^m2
    Ws2[:, 0] = (-1.0) ** np.arange(512)

    def dev_w(Wmf):  # [512 m2, 1024 fa] -> [128 p, NFC, NMC2, 128 fi]
        a = Wmf.T.reshape(NFC, 128, NMC2, 128)  # [chunk, fi, mc2, p]
        return np.ascontiguousarray(a.transpose(3, 0, 2, 1)).astype(
            ml_dtypes.bfloat16
        )

    wc_dev = dev_w(Wc2)
    ws_dev = dev_w(Ws2)

    # mel columns in arranged-f order; f=0 slot becomes mel_w[:,1024]
    # (applied to C_0^2 + S_nyq^2); the rank-1
    # (mel_w[:,0]-mel_w[:,1024]) x C_0^2 term repairs it
    mel_mod = mel[:, :1024].copy()
    mel_mod[:, 0] = mel[:, 1024]
    melP = mel_mod[:, fs]  # [NMEL, 1024 arranged]
    melT = np.ascontiguousarray(
        melP.T.reshape(NFC, 128, NMEL).transpose(1, 0, 2)
    ).astype(ml_dtypes.bfloat16)  # [128 fi, NFC, NMEL]
    melnyq = np.ascontiguousarray((mel[:, 0] - mel[:, 1024])[None, :]).astype(
        ml_dtypes.bfloat16
    )  # [1, NMEL]
    return c2, c2r, wc_dev, ws_dev, wv, wrv, melT, melnyq


def _make_in_maps(inputs):
    c2, c2r, wc_dev, ws_dev, wv, wrv, melT, melnyq = _prep_inputs(**inputs)
    in_maps = []
    for c in range(NCORES):
        in_maps.append(
            {
                "c2": c2[c * BPC : (c + 1) * BPC],
                "c2r": c2r[c * BPC : (c + 1) * BPC],
                "wc": wc_dev,
                "ws": ws_dev,
                "wv": wv,
                "wrv": wrv,
                "melT": melT,
                "melnyq": melnyq,
            }
        )
    return in_maps


def kernel(x, cos_w, sin_w, mel_w):
    nc = _get_nc()
    in_maps = _make_in_maps(
        {"x": x, "cos_w": cos_w, "sin_w": sin_w, "mel_w": mel_w}
    )
    res = run_bass_kernel_spmd(nc, in_maps, list(range(NCORES)))
    out = np.concatenate(
        [
            np.concatenate([r["out1"], r["out2"]], axis=2)
            for r in res.results
        ],
        axis=0,
    )  # [32,128,857]
    return out.astype(np.float32)


if __name__ == "__main__":
    rng = np.random.default_rng(0)
    x = rng.standard_normal((B, 1, T), dtype=np.float32)
    wc = rng.standard_normal((1025, 1, WIN), dtype=np.float32)
    wsn = rng.standard_normal((1025, 1, WIN), dtype=np.float32)
    mw = np.abs(rng.standard_normal((NMEL, 1025), dtype=np.float32)).astype(np.float32)
    o = kernel(x, wc, wsn, mw)
    print(o.shape, o.dtype)


# revision 37
# speedup vs baseline: 1.4530x; 1.0213x over previous
"""MelSpectrogramNet on 8 TRN2 NeuronCores (Bass/Tile), data-parallel over batch.

Math (per batch item):
  stft[f,t]  = (sum_k x[256t+k]*wc[f,k])^2 + (sum_k x[256t+k]*ws[f,k])^2
  mel        = mel_w @ stft
  x_db       = 10*log10(max(mel, 1e-10));  x_db = max(x_db, max_all(x_db)-80)
  out        = (x_db + 25) / 80

Folded DFT (the key trick): the hann window is exactly symmetric
(w[k] = w[2047-k]), so with j = k - 1023.5 the windowed DFT row is
w*cos(theta_f*j + phi_f). Folding x about the window center into
  e_m(t) = x[256t+1024+m] + x[256t+1023-m]
  o_m(t) = x[256t+1024+m] - x[256t+1023-m]        (m in [0,1024))
gives  cosDFT = cos(phi)C - sin(phi)S,  sinDFT = sin(phi)C + cos(phi)S with
  C_f = sum_m W~c[m,f] e_m,   S_f = sum_m W~s[m,f] o_m
and the power is phi-free:  power_f = C_f^2 + S_f^2.
=> the tensor-engine contraction halves (K=1024 per transform instead of
2x K=2048), which matters because the PE is the bottleneck (GPIO power
throttle caps it at 13/16 duty; the f32r version already ran at ~96% of
that throttled roofline).

Device mapping:
  - x is de-interleaved by 128-column parity into C2[r, par, u] =
    x[256u+128par+r] plus a partition-reversed copy C2R[r,...] =
    C2[127-r,...]; the DVE then computes each 128-row m-chunk of e/o as a
    single tensor_tensor add/sub of two contiguous slices (hidden under
    the matmuls of the previous tile).
  - all matmul operands are bf16 (measured end-to-end rel err ~5e-3 vs
    the 2e-2 gate); PSUM accumulation is fp32.
  - Nyquist: C_1024 = 0 exactly and the S-weight column for f=0 is exactly
    zero, so the S weights carry w~*(-1)^m (the Nyquist sine row) in the
    f=0 slot. Then stft[0] = C_0^2 + S_nyq^2; the mel weight column for
    f=0 is swapped to mel_w[:,1024] and a K=1 rank-1 matmul with
    (mel_w[:,0]-mel_w[:,1024]) x C_0^2 repairs the difference.
  - top_db clamp in linear space: pass 1 keeps out_pre in SBUF and the
    per-core max of mel; after gpsimd partition_all_reduce +
    AllReduce(max), the fixup applies out = max(out_pre, o_thr) in-place
    and DMAs straight out — no DRAM round-trip in the tail.
"""
import sys

sys.path.insert(0, "/opt/trn_rl_repo")

import ml_dtypes
import numpy as np

from concourse import bacc, bass_isa, mybir, tile
from concourse.bass_utils import run_bass_kernel_spmd

dt = mybir.dt
AF = mybir.ActivationFunctionType
ALU = mybir.AluOpType

NCORES = 8
B, T = 32, 221184
WIN, HOP = 2048, 256
FRAMES = (T - WIN) // HOP + 1  # 857
NMEL = 128
BPC = B // NCORES  # 4
UCOLS = T // 256  # 864 columns of 128 per parity
NFC = 8  # f-chunks of 128 (f = 0..1023); f=1024 (Nyquist) folded into S f=0
NMC = 8  # m-chunks of 128 (folded window half, m = 0..1023)
NMC2 = 4  # m-chunks after the SECOND fold (m = 0..511)
# Second tile overlaps the first by 3 frames so its width is a multiple of 4;
# overlapped frames are recomputed with identical values.
T_TILES = [(0, 512), (FRAMES - 348, 348)]
FIX_TILES = [(0, 512), (512, FRAMES - 512)]  # non-overlapping, for the fixup
C_LOG = 10.0 / float(np.log(10.0))  # 10*log10(x) = C_LOG * ln(x)
AMIN = 1e-10
TOPDB_LIN = 1e-8  # 10**(-80/10)

_compiled = {}


def _build_nc():
    nc = bacc.Bacc(
        "TRN2", target_bir_lowering=False, debug=False, num_devices=NCORES
    )

    c2_d = nc.dram_tensor("c2", [BPC, 128, 2, UCOLS], dt.bfloat16, kind="ExternalInput")
    c2r_d = nc.dram_tensor(
        "c2r", [BPC, 128, 2, UCOLS], dt.bfloat16, kind="ExternalInput"
    )
    wc_d = nc.dram_tensor("wc", [128, NFC, NMC2, 128], dt.bfloat16, kind="ExternalInput")
    ws_d = nc.dram_tensor("ws", [128, NFC, NMC2, 128], dt.bfloat16, kind="ExternalInput")
    wv_d = nc.dram_tensor("wv", [128, NMC], dt.float32, kind="ExternalInput")
    wrv_d = nc.dram_tensor("wrv", [128, NMC], dt.float32, kind="ExternalInput")
    melT_d = nc.dram_tensor("melT", [128, NFC, NMEL], dt.bfloat16, kind="ExternalInput")
    melnyq_d = nc.dram_tensor("melnyq", [1, NMEL], dt.bfloat16, kind="ExternalInput")
    out1_d = nc.dram_tensor("out1", [BPC, NMEL, 512], dt.bfloat16, kind="ExternalOutput")
    out2_d = nc.dram_tensor(
        "out2", [BPC, NMEL, FRAMES - 512], dt.bfloat16, kind="ExternalOutput"
    )

    with tile.TileContext(nc) as tc:
        with (
            tc.tile_pool(name="sbw", bufs=1) as sbw,
            tc.tile_pool(name="sbeo", bufs=2) as sbeo,
            tc.tile_pool(name="sbe", bufs=3) as sbe,
            tc.tile_pool(name="sbf", bufs=8) as sbf,
            tc.tile_pool(name="sbt", bufs=2) as sbt,
            tc.tile_pool(name="psCS", bufs=3, space="PSUM") as psCS,
            tc.tile_pool(name="psM", bufs=2, space="PSUM") as psM,
            tc.tile_pool(name="dram", bufs=1, space="DRAM") as dram,
        ):
            # persistent SBUF tensors
            c2s, c2rs, outp = [], [], []
            for b in range(BPC):
                c2s.append(sbw.tile([128, 2, UCOLS], dt.bfloat16, name=f"c2_{b}"))
                c2rs.append(sbw.tile([128, 2, UCOLS], dt.bfloat16, name=f"c2r_{b}"))
                outp.append(sbw.tile([128, FRAMES], dt.float32, name=f"outp_{b}"))
            wc_t = [sbw.tile([128, NMC2, 128], dt.bfloat16, name=f"wc{fc}") for fc in range(NFC)]
            ws_t = [sbw.tile([128, NMC2, 128], dt.bfloat16, name=f"ws{fc}") for fc in range(NFC)]
            wv_t = sbw.tile([128, NMC], dt.float32, name="wv_t")
            wrv_t = sbw.tile([128, NMC], dt.float32, name="wrv_t")
            melT_t = sbw.tile([128, NFC, NMEL], dt.bfloat16, name="melT_t")
            melnyq_t = sbw.tile([1, NMEL], dt.bfloat16, name="melnyq_t")
            nslots = BPC * len(T_TILES)
            maxslots = sbw.tile([128, nslots], dt.float32, name="maxslots")


            # ---- input DMAs: b=0 slices needed by the first tile go first.
            # fold mc=0 needs c2 parity 0 + c2r parity 1, so those two land
            # first.
            nc.gpsimd.dma_start(c2s[0][:, 0, 0:520], c2_d.ap()[0][:, 0, 0:520])
            nc.sync.dma_start(c2rs[0][:, 1, 0:520], c2r_d.ap()[0][:, 1, 0:520])
            nc.gpsimd.dma_start(c2s[0][:, 1, 0:520], c2_d.ap()[0][:, 1, 0:520])
            nc.gpsimd.dma_start(c2rs[0][:, 0, 0:520], c2r_d.ap()[0][:, 0, 0:520])
            # fc=0/1 weights split across sync/scalar queues so the first
            # matmuls are never DMA-starved; melT's first chunks land early
            # (the mel matmul is on the in-order PE queue — starving it
            # stalls the PE), then the remaining f-chunks alternate queues.
            nc.sync.dma_start(wv_t[:], wv_d.ap())
            nc.sync.dma_start(wrv_t[:], wrv_d.ap())
            nc.sync.dma_start(wc_t[0][:, 0:2], wc_d.ap()[:, 0, 0:2])
            nc.scalar.dma_start(wc_t[0][:, 2:], wc_d.ap()[:, 0, 2:])
            nc.sync.dma_start(ws_t[0][:, 0:2], ws_d.ap()[:, 0, 0:2])
            nc.scalar.dma_start(ws_t[0][:, 2:], ws_d.ap()[:, 0, 2:])
            nc.sync.dma_start(melnyq_t[:], melnyq_d.ap())
            nc.sync.dma_start(melT_t[:, 0:2], melT_d.ap()[:, 0:2])
            nc.sync.dma_start(wc_t[1][:], wc_d.ap()[:, 1])
            nc.scalar.dma_start(ws_t[1][:], ws_d.ap()[:, 1])

            # Warm up the collective engine while the DFT runs so the real
            # AllReduce at the end starts with rings already configured.
            ccw_in = dram.tile([1, 128], dt.float32, name="ccw_in")
            ccw_out = dram.tile([1, 128], dt.float32, name="ccw_out")
            nc.gpsimd.collective_compute(
                "AllReduce",
                ALU.max,
                replica_groups=[list(range(NCORES))],
                ins=[ccw_in[:].opt()],
                outs=[ccw_out[:].opt()],
            )

            nc.sync.dma_start(wc_t[2][:], wc_d.ap()[:, 2])
            nc.scalar.dma_start(ws_t[2][:], ws_d.ap()[:, 2])
            nc.sync.dma_start(melT_t[:, 2:], melT_d.ap()[:, 2:])
            for fc in range(3, NFC):
                nc.sync.dma_start(wc_t[fc][:], wc_d.ap()[:, fc])
                nc.scalar.dma_start(ws_t[fc][:], ws_d.ap()[:, fc])
            nc.gpsimd.dma_start(c2s[0][:, :, 520:], c2_d.ap()[0][:, :, 520:])
            nc.gpsimd.dma_start(c2rs[0][:, :, 520:], c2r_d.ap()[0][:, :, 520:])
            for b in range(1, BPC):
                nc.gpsimd.dma_start(c2s[b][:], c2_d.ap()[b])
                nc.gpsimd.dma_start(c2rs[b][:], c2r_d.ap()[b])

            # ---- pass 1: fold + folded DFT power + mel + log/affine ----
            slots = [(b, t0, tt) for b in range(BPC) for t0, tt in T_TILES]
            # even m-chunks need (c2 par0, c2r par1); odds the other pair —
            # process evens first so the first matmuls match DMA arrival order
            MC_ORDER = [0, 2, 4, 6, 1, 3, 5, 7]

            def emit_fold(si):
                # DVE fold: e/o m-chunks as adds/subs of shifted slices.
                # Slot 0 folds in 256-wide halves so work can start as soon
                # as the first x quarters land (keeps early PE gaps under
                # the ~3.4us HAM re-throttle window).
                # DVE double-fold. First fold (window-center symmetry):
                #   e_m = x[256t+1024+m] + x[256t+1023-m]   (m in [0,1024))
                # then window ON THE DATA (ew = w~*e) and fold again about
                # m <-> 1023-m (cos kernel parity: the mirrored half enters
                # with sign (-1)^f):
                #   E+/-_m = ew_m +/- ew_{1023-m}           (m in [0,512))
                # and analogously for the sine side. Even frequencies use
                # (E+, O-), odd use (E-, O+): contraction K halves to 512.
                b, t0, tt = slots[si]
                EP = sbeo.tile([128, NMC2, tt], dt.bfloat16, tag="EP")
                EM = sbeo.tile([128, NMC2, tt], dt.bfloat16, tag="EM")
                OP = sbeo.tile([128, NMC2, tt], dt.bfloat16, tag="OP")
                OM = sbeo.tile([128, NMC2, tt], dt.bfloat16, tag="OM")
                for mc in range(NMC2):
                    # direct half (m = 128mc+p): x[256t+1024+m], x[256t+1023-m]
                    u1 = t0 + 4 + mc // 2
                    u2 = t0 + 3 - mc // 2
                    a = c2s[b][:, mc % 2, u1 : u1 + tt]
                    r = c2rs[b][:, 1 - mc % 2, u2 : u2 + tt]
                    # mirrored half (m~ = 1023-m): x[256t+2047-m], x[256t+m]
                    q = 15 - mc
                    u3 = t0 + q // 2
                    u4 = t0 + mc // 2
                    am = c2rs[b][:, q % 2, u3 : u3 + tt]
                    rm = c2s[b][:, mc % 2, u4 : u4 + tt]
                    rho = wv_t[:, mc : mc + 1]
                    rhon = wrv_t[:, mc : mc + 1]
                    ev = sbt.tile([128, tt], dt.bfloat16, tag="ew")
                    emir = sbt.tile([128, tt], dt.bfloat16, tag="rw")
                    nc.vector.tensor_tensor(ev[:], a, r, ALU.add)
                    nc.vector.tensor_tensor(emir[:], am, rm, ALU.add)
                    nc.vector.scalar_tensor_tensor(
                        EP[:, mc], emir[:], rho, ev[:], ALU.mult, ALU.add
                    )
                    nc.vector.scalar_tensor_tensor(
                        EM[:, mc], emir[:], rhon, ev[:], ALU.mult, ALU.add
                    )
                    ov = sbt.tile([128, tt], dt.bfloat16, tag="ow")
                    omir = sbt.tile([128, tt], dt.bfloat16, tag="row")
                    nc.vector.tensor_tensor(ov[:], a, r, ALU.subtract)
                    nc.vector.tensor_tensor(omir[:], am, rm, ALU.subtract)
                    # even f: O- = o - rho*omir ; odd f: O+ = o + rho*omir
                    nc.vector.scalar_tensor_tensor(
                        OM[:, mc], omir[:], rhon, ov[:], ALU.mult, ALU.add
                    )
                    nc.vector.scalar_tensor_tensor(
                        OP[:, mc], omir[:], rho, ov[:], ALU.mult, ALU.add
                    )
                return (EP, EM, OP, OM)

            def emit_epilogue(slot, defer=None):
                # mel -> per-slot max -> clamp(AMIN) -> ln -> affine -> outp
                b, t0, tt = slots[slot]
                mel_ps = mel_pss[slot]
                mel_sb = sbe.tile([128, tt], dt.float32, tag="melsb")
                nc.vector.tensor_reduce(
                    maxslots[:, slot : slot + 1], mel_ps[:],
                    mybir.AxisListType.X, ALU.max,
                )
                if defer is not None:
                    defer()  # last slot: thr chain ahead of the Ln/affine
                nc.vector.tensor_scalar(mel_sb[:], mel_ps[:], AMIN, None, ALU.max)
                nc.scalar.activation(mel_sb[:], mel_sb[:], AF.Ln)
                nc.vector.tensor_scalar(
                    outp[b][:, t0 : t0 + tt], mel_sb[:],
                    C_LOG / 80.0, 25.0 / 80.0, ALU.mult, ALU.add,
                )

            eo_next = emit_fold(0)
            mel_pss = {}
            for slot, (b, t0, tt) in enumerate(slots):
                EPc, EMc, OPc, OMc = eo_next
                mel_ps = psM.tile([128, tt], dt.float32, tag="mel")
                mel_pss[slot] = mel_ps
                # mel matmuls are emitted one fc-iteration late so the
                # in-order PE queue never waits on the Square/add chain;
                # (stft tile, fc) pending between iterations:
                pend = None
                for fc in range(NFC):
                    if fc == 4 and slot + 1 < len(slots):
                        # software-pipeline: fold the next slot's e/o now so
                        # the PE never waits on the DVE at slot boundaries
                        eo_next = emit_fold(slot + 1)
                    c_ps = psCS.tile([128, tt], dt.float32, tag="C")
                    s_ps = psCS.tile([128, tt], dt.float32, tag="S")
                    cmov = EPc if fc < 4 else EMc
                    smov = OMc if fc < 4 else OPc
                    for mc in range(NMC2):
                        nc.tensor.matmul(
                            c_ps[:], wc_t[fc][:, mc, :], cmov[:, mc],
                            start=(mc == 0), stop=(mc == NMC2 - 1),
                            skip_group_check=True,
                        )
                    for mc in range(NMC2):
                        nc.tensor.matmul(
                            s_ps[:], ws_t[fc][:, mc, :], smov[:, mc],
                            start=(mc == 0), stop=(mc == NMC2 - 1),
                            skip_group_check=True,
                        )
                    if fc == 1:
                        # rank-1 repair of the Nyquist fold (see header);
                        # first write of mel_ps (start=True)
                        nc.tensor.matmul(
                            mel_ps[:], melnyq_t[:], prev_csq[0:1, :],
                            start=True, stop=False, skip_group_check=True,
                        )
                    if pend is not None:
                        pstft, pfc = pend
                        nc.tensor.matmul(
                            mel_ps[:], melT_t[:, pfc, :], pstft[:],
                            start=False, stop=False, skip_group_check=True,
                        )
                    csq = sbe.tile([128, tt], dt.bfloat16, tag="csq")
                    ssq = sbe.tile([128, tt], dt.bfloat16, tag="ssq")
                    nc.scalar.activation(csq[:], c_ps[:], AF.Square)
                    nc.scalar.activation(ssq[:], s_ps[:], AF.Square)
                    if fc == 0:
                        prev_csq = csq
                    stft = sbe.tile([128, tt], dt.bfloat16, tag="stft")
                    nc.vector.tensor_tensor(stft[:], csq[:], ssq[:], ALU.add)
                    pend = (stft, fc)
                    if fc == 1 and slot > 0:
                        # previous slot's last mel matmul + epilogue, emitted
                        # here so its Square/add chain hides under this
                        # slot's DFT matmuls
                        lstft, lfc = last_pend
                        nc.tensor.matmul(
                            mel_pss[slot - 1][:], melT_t[:, lfc, :], lstft[:],
                            start=False, stop=True, skip_group_check=True,
                        )
                        emit_epilogue(slot - 1)
                last_pend = pend

            # last slot: flush the final mel matmul + epilogue directly
            lstft, lfc = last_pend
            nc.tensor.matmul(
                mel_pss[len(slots) - 1][:], melT_t[:, lfc, :], lstft[:],
                start=False, stop=True, skip_group_check=True,
            )
            # ---- local threshold, then AllReduce(max) of the threshold ----
            # The dB transform is monotone increasing, so
            # max_c f(lmax_c) == f(max_c lmax_c): compute the local o_thr
            # BEFORE the collective, and emit the whole chain ahead of the
            # last slot's Ln/affine so the collective triggers ASAP.
            cc_in = dram.tile([1, 128], dt.float32, name="cc_in")
            cc_out = dram.tile([1, 128], dt.float32, name="cc_out")

            def emit_thr_chain():
                lmax = sbw.tile([128, 1], dt.float32, name="lmax")
                nc.vector.tensor_reduce(
                    lmax[:], maxslots[:], mybir.AxisListType.X, ALU.max
                )
                gmax = sbw.tile([128, 1], dt.float32, name="gmax")
                nc.gpsimd.partition_all_reduce(
                    gmax[:], lmax[:], channels=128, reduce_op=bass_isa.ReduceOp.max
                )
                # ln(gmax * 1e-8) in one activation (scale folds the mult)
                thrln = sbw.tile([128, 1], dt.float32, name="thrln")
                nc.scalar.activation(thrln[:], gmax[:], AF.Ln, scale=TOPDB_LIN)
                lthr = sbw.tile([128, 1], dt.float32, name="lthr")
                nc.vector.tensor_scalar(
                    lthr[:], thrln[:], C_LOG / 80.0, 25.0 / 80.0, ALU.mult, ALU.add
                )
                # on the gpsimd queue: the collective trigger is also on
                # gpsimd, so no cross-engine semaphore handoff before it
                nc.gpsimd.dma_start(cc_in[:], lthr[:])
                nc.gpsimd.collective_compute(
                    "AllReduce",
                    ALU.max,
                    replica_groups=[list(range(NCORES))],
                    ins=[cc_in[:].opt()],
                    outs=[cc_out[:].opt()],
                )

            emit_epilogue(len(slots) - 1, defer=emit_thr_chain)
            o_thr = sbw.tile([128, 1], dt.float32, name="o_thr")
            nc.sync.dma_start(o_thr[:], cc_out[:])

            # ---- fixup: out = max(out_pre, o_thr), in-place, then DMA out ----
            qs = [nc.sync, nc.scalar]
            i = 0
            for b in range(BPC):
                for (t0, tt), od in zip(FIX_TILES, (out1_d, out2_d)):
                    oc = sbf.tile([128, tt], dt.bfloat16, tag="oc")
                    nc.vector.tensor_scalar(
                        oc[:], outp[b][:, t0 : t0 + tt], o_thr[:], None, ALU.max
                    )
                    qs[i % 2].dma_start(od.ap()[b], oc[:])
                    i += 1

    nc.compile()
    return nc


def _get_nc():
    if "nc" not in _compiled:
        _compiled["nc"] = _build_nc()
    return _compiled["nc"]


def _prep_inputs(x, cos_w, sin_w, mel_w):
    x = np.asarray(x, dtype=np.float32).reshape(B, T)
    wcf = np.asarray(cos_w, dtype=np.float32).reshape(WIN // 2 + 1, WIN)  # [1025,2048]
    wsf = np.asarray(sin_w, dtype=np.float32).reshape(WIN // 2 + 1, WIN)
    mel = np.asarray(mel_w, dtype=np.float32)  # [128, 1025]

    # x -> [B, 128, 2, 864]: C2[r, par, u] = x[256u + 128par + r], bf16,
    # plus the partition-reversed copy for the fold's mirrored operand.
    x16 = x.astype(ml_dtypes.bfloat16)
    c2 = np.ascontiguousarray(x16.reshape(B, UCOLS, 2, 128).transpose(0, 3, 2, 1))
    c2r = np.ascontiguousarray(c2[:, ::-1])

    # Double-fold prep. The window (applied on-chip to the folded data)
    # comes from the provided cos_w f=0 row, which is exactly hann:
    wfull = wcf[0, 1024:].astype(np.float64)  # [1024] = hann[1024:]
    # rho[p, mc] = w~[1023-m]/w~[m] (m = 128mc+p, m < 512; bounded <= 1):
    # the mirrored half of the second fold enters as e +/- rho*e_mirror so
    # the weights stay windowed and only ONE scale is applied on-chip.
    mhalf = np.arange(512)
    rho_full = wfull[1023 - mhalf] / wfull[mhalf]
    wv = np.zeros((128, NMC), np.float32)
    wv[:, :NMC2] = rho_full.reshape(NMC2, 128).T
    wrv = -wv

    # Pure-trig second-fold weights; frequencies regrouped by parity:
    # chunks 0-3 = even f (0,2,..,1022), chunks 4-7 = odd f (1,3,..,1023).
    m2 = np.arange(512, dtype=np.float64)
    j2 = m2 + 0.5
    fs = np.concatenate([2 * np.arange(512), 2 * np.arange(512) + 1])
    th = 2.0 * np.pi * fs[None, :].astype(np.float64) / WIN
    Wc2 = wfull[:512, None] * np.cos(th * j2[:, None])  # windowed, [512, 1024]
    Ws2 = wfull[:512, None] * np.sin(th * j2[:, None])
    # arranged position 0 is f=0 whose S row is exactly zero; carry the
    # Nyquist S row there: w~ * sin(pi*(m2+0.5)) = w~ * (-1)^m2
    Ws2[:, 0] = wfull[:512] * ((-1.0) ** np.arange(512))

    def dev_w(Wmf):  # [512 m2, 1024 fa] -> [128 p, NFC, NMC2, 128 fi]
        a = Wmf.T.reshape(NFC, 128, NMC2, 128)  # [chunk, fi, mc2, p]
        return np.ascontiguousarray(a.transpose(3, 0, 2, 1)).astype(
            ml_dtypes.bfloat16
        )

    wc_dev = dev_w(Wc2)
    ws_dev = dev_w(Ws2)

    # mel columns in arranged-f order; f=0 slot becomes mel_w[:,1024]
    # (applied to C_0^2 + S_nyq^2); the rank-1
    # (mel_w[:,0]-mel_w[:,1024]) x C_0^2 term repairs it
    mel_mod = mel[:, :1024].copy()
    mel_mod[:, 0] = mel[:, 1024]
    melP = mel_mod[:, fs]  # [NMEL, 1024 arranged]
    melT = np.ascontiguousarray(
        melP.T.reshape(NFC, 128, NMEL).transpose(1, 0, 2)
    ).astype(ml_dtypes.bfloat16)  # [128 fi, NFC, NMEL]
    melnyq = np.ascontiguousarray((mel[:, 0] - mel[:, 1024])[None, :]).astype(
        ml_dtypes.bfloat16
    )  # [1, NMEL]
    return c2, c2r, wc_dev, ws_dev, wv, wrv, melT, melnyq


def _make_in_maps(inputs):
    c2, c2r, wc_dev, ws_dev, wv, wrv, melT, melnyq = _prep_inputs(**inputs)
    in_maps = []
    for c in range(NCORES):
        in_maps.append(
            {
                "c2": c2[c * BPC : (c + 1) * BPC],
                "c2r": c2r[c * BPC : (c + 1) * BPC],
                "wc": wc_dev,
                "ws": ws_dev,
                "wv": wv,
                "wrv": wrv,
                "melT": melT,
                "melnyq": melnyq,
            }
        )
    return in_maps


def kernel(x, cos_w, sin_w, mel_w):
    nc = _get_nc()
    in_maps = _make_in_maps(
        {"x": x, "cos_w": cos_w, "sin_w": sin_w, "mel_w": mel_w}
    )
    res = run_bass_kernel_spmd(nc, in_maps, list(range(NCORES)))
    out = np.concatenate(
        [
            np.concatenate([r["out1"], r["out2"]], axis=2)
            for r in res.results
        ],
        axis=0,
    )  # [32,128,857]
    return out.astype(np.float32)


if __name__ == "__main__":
    rng = np.random.default_rng(0)
    x = rng.standard_normal((B, 1, T), dtype=np.float32)
    wc = rng.standard_normal((1025, 1, WIN), dtype=np.float32)
    wsn = rng.standard_normal((1025, 1, WIN), dtype=np.float32)
    mw = np.abs(rng.standard_normal((NMEL, 1025), dtype=np.float32)).astype(np.float32)
    o = kernel(x, wc, wsn, mw)
    print(o.shape, o.dtype)
